# revision 83
# baseline (speedup 1.0000x reference)
"""Trainium2 Bass kernel for nn_Half_Graph (GNN message passing block).

Data-parallel over batch: core b processes image b (B=8 across 8 cores).

Planar G=6 layout: partition 10*g + c <-> (channel c, pixel group g) over
6 groups of 6144 pixels; upper/lower halves live at partition offsets
0/64. All image traffic is bf16 and pair-packed (DMA cost is free-bytes
per partition, so tall 128-row transfers are the cheap shape).

Each 1x1 conv runs as ONE tensor-engine pass per (conv, input tile):
conv1 concat operands are assembled in SBUF regions ([premult ; xh]
column blocks) — premults write one 64-row half, the xh half arrives via
a replicated DMA (U) or 4x-mode bf16 DVE copies (L) — so conv1 needs 1
pass instead of 2. The p_att sum+broadcast rides a ones-stationary
matmul; the single h_att planes are host-expanded (layout only) and
loaded directly. The z biases ride a ones-row in H (pad row 60, produced
free by the H relu bias) into the conv2 stationaries, so the two
single-block z relus fuse into the message adds as (psum max 0) add acc
scalar_tensor_tensor ops. The GRU gates read [msg, xh] via split
stationaries (no concat); GRU output is out = h + u * (c - h).

Engine placement respects HW limits (GPSIMD cannot touch PSUM; all
TensorTensor operands must share a start partition): PSUM evacuations
go to Act (sigmoid/tanh/relu) and DVE (tensor_scalar), SBUF-only
elementwise work goes to Pool and DVE's 2x/4x bf16 modes. The chunk
loop is software-pipelined 4 deep (P1 loads+att / P2 premults / C convs
/ B1 r,u gates / B2 candidate+combine+store) so each in-order engine
queue only sees ready work.
"""

import sys

for _p in ("/opt/trn_rl_repo", "/root/.axon_site/_ro/trn_rl_repo"):
    if _p not in sys.path:
        sys.path.insert(0, _p)

import numpy as np

import concourse.bass as bass
import concourse.bacc as bacc
import concourse.mybir as mybir
from concourse.tile import TileContext

F32 = mybir.dt.float32
BF16 = mybir.dt.bfloat16
AL = mybir.AluOpType
AF = mybir.ActivationFunctionType

B = 8
HD = 10
HW = 192 * 192
G = 6
GP = HW // G            # 6144
CW = 1024
NCHUNK = GP // CW       # 6
EPS = 1e-5
H1 = 64

# stationary indices
(S_UL, S_A12, S_DU, S_DL, S_CU, S_CL,
 S_DW2H0, S_DW2H1, S_UW2H0, S_UW2H1, S_LW2H0, S_LW2H1,
 S_R_M, S_R_H, S_U_M, S_U_H, S_C_M, S_C_RH) = range(18)
NS = 18

# bias vector indices (BV_ZS: upper-only z bias, lower rows zero)
(BV_D1, BV_U1, BV_L1, BV_Z0, BV_Z1, BV_ZS, BV_R, BV_U, BV_C) = range(9)
NB = 9

# comp block order: (block, z index, role). role 0/1 = first/second pass of
# an (upper, lower) psum pair; role 2 = single upper-only psum (blocks 2, 3
# have no lower partner; stationary zero-fills cols 64:124).
BLOCK_SEQ = [(0, 1, 0), (4, 1, 1), (1, 2, 0), (5, 2, 1), (2, 3, 2), (3, 4, 2)]
W2_STAT = {(True, 0): S_UW2H0, (True, 1): S_UW2H1,
           (False, 0): S_LW2H0, (False, 1): S_LW2H1}


def _build_nc():
    nc = bacc.Bacc(trn_type="TRN2")

    xh = nc.declare_dram_parameter("xh", [128, GP], BF16, isOutput=False)
    # per chunk: [ [xp0;xp4] | [xp1;xp5] ] column pairs
    xp0415 = nc.declare_dram_parameter("xp0415", [128, 2 * GP], BF16,
                                       isOutput=False)
    # per chunk: [ xp2 | xp3 ] at rows 0:64 (both feed upper blocks)
    xp23r = nc.declare_dram_parameter("xp23r", [64, 2 * GP], BF16,
                                      isOutput=False)
    xf2 = nc.declare_dram_parameter("xf2", [128, GP], BF16, isOutput=False)
    # xh upper half 5x-replicated per chunk (fills U rows 64:128 in one DMA)
    xhu5 = nc.declare_dram_parameter("xhu5", [64, 5 * GP], BF16,
                                     isOutput=False)
    att = nc.declare_dram_parameter("att", [36, GP], BF16, isOutput=False)
    # host-expanded h_att planes: rows 0:60 = a1 per-group bcast, 64:124 = a2
    a12e = nc.declare_dram_parameter("a12e", [128, GP], BF16, isOutput=False)
    smats = nc.declare_dram_parameter("smats", [128, NS * 128], BF16,
                                      isOutput=False)
    bvecs = nc.declare_dram_parameter("bvecs", [128, NB], F32, isOutput=False)
    out = nc.declare_dram_parameter("out", [128, GP], BF16, isOutput=True)

    with TileContext(nc) as tc:
        with (
            tc.tile_pool(name="const", bufs=1) as cpool,
            tc.tile_pool(name="ld", bufs=3) as ld,
            tc.tile_pool(name="reg", bufs=2) as reg,
            tc.tile_pool(name="ae", bufs=2) as ae,
            tc.tile_pool(name="hp", bufs=2) as hp,
            tc.tile_pool(name="zp", bufs=2) as zp,
            tc.tile_pool(name="gp", bufs=2) as gpl,
            tc.tile_pool(name="psum", bufs=4, space="PSUM") as pp,
        ):
            smt = cpool.tile([128, NS * 128], BF16)
            nc.sync.dma_start(out=smt[:, :], in_=smats[:, :])
            bv = cpool.tile([128, NB], F32)
            nc.sync.dma_start(out=bv[:, :], in_=bvecs[:, :])
            attT = cpool.tile([36, GP], BF16)
            nc.sync.dma_start(out=attT[:, 0:CW], in_=att[:, 0:CW])

            def stat(i, K, base=0):
                return smt[base:base + K, i * 128:(i + 1) * 128]

            def mm(ps, s_idx, K, rhs_ap, start, stop, base=0):
                lhsT = stat(s_idx, K, base)
                for s in range(0, CW, 512):
                    nc.tensor.matmul(ps[0:128, s:s + 512], lhsT,
                                     rhs_ap[:, s:s + 512],
                                     start=start, stop=stop)

            def bias(k):
                return bv[0:128, k:k + 1]

            def stageP1(j):
                c0, c1 = j * CW, (j + 1) * CW

                # ---------------- loads (bf16, SP) ----------------
                xhP = ld.tile([128, CW], BF16, tag="xhP", bufs=5)
                nc.sync.dma_start(out=xhP[:, :], in_=xh[:, c0:c1])
                xp0415t = ld.tile([128, 2 * CW], BF16, tag="xp0415", bufs=3)
                nc.sync.dma_start(out=xp0415t[:, :],
                                  in_=xp0415[:, 2 * c0:2 * c1])
                xp23rt = ld.tile([64, 2 * CW], BF16, tag="xp23r", bufs=3)
                nc.sync.dma_start(out=xp23rt[:, :], in_=xp23r[:, 2 * c0:2 * c1])
                xf2t = ld.tile([128, CW], BF16, tag="xf2", bufs=3)
                nc.sync.dma_start(out=xf2t[:, :], in_=xf2[:, c0:c1])
                attE12 = ld.tile([128, CW], BF16, tag="attE12", bufs=3)
                nc.sync.dma_start(out=attE12[:, :], in_=a12e[:, c0:c1])

                # attention broadcast (PE, first matmuls of each cycle) + evac
                p_ul = pp.tile([128, CW], F32, tag="ps", name="p_ul")
                mm(p_ul, S_UL, 36, attT[0:36, c0:c1], True, True)
                attE = ae.tile([128, CW], BF16, tag="attE", bufs=3)
                nc.vector.tensor_copy(attE[:, :], p_ul[:, :])
                return dict(xhP=xhP, xp0415t=xp0415t, xp23rt=xp23rt,
                            xf2t=xf2t, attE12=attE12, attE=attE,
                            c0=c0, c1=c1, j=j)

            def stageP2(p1):
                j = p1["j"]
                xhP, xp0415t, xp23rt = p1["xhP"], p1["xp0415t"], p1["xp23rt"]
                xf2t, attE12, attE = p1["xf2t"], p1["attE12"], p1["attE"]
                c0, c1 = p1["c0"], p1["c1"]

                # concat regions: U blocks [dec_u, c0..c3], L [dec_l, c4, c5]
                U = reg.tile([128, 5 * CW], BF16, tag="U")
                L = reg.tile([128, 3 * CW], BF16, tag="L")

                # xh placements: U bottoms via one replicated DMA (SP has
                # headroom) — except chunk 0, where SP is the startup critical
                # path and DVE is idle; L tops via DVE 4x bf16 copies
                if j <= 1:
                    for k in range(5):
                        nc.vector.tensor_copy(U[64:128, k * CW:(k + 1) * CW],
                                              xhP[0:64, :])
                else:
                    nc.sync.dma_start(out=U[64:128, :],
                                      in_=xhu5[:, j * 5 * CW:(j + 1) * 5 * CW])
                for k in range(3):
                    nc.vector.tensor_copy(L[0:64, k * CW:(k + 1) * CW],
                                          xhP[64:128, :])

                # ------------- premultiplies (Pool/DVE bf16) -------------
                # all operands share a start partition (HW BIR requirement)
                nc.vector.tensor_tensor(U[0:64, 0:CW], xf2t[0:64, :],
                                        attE12[0:64, :], AL.mult)
                nc.gpsimd.tensor_tensor(L[64:128, 0:CW], xf2t[64:128, :],
                                        attE12[64:128, :], AL.mult)
                nc.gpsimd.tensor_tensor(U[0:64, CW:2 * CW],
                                        xp0415t[0:64, 0:CW],
                                        attE[0:64, :], AL.mult)
                nc.gpsimd.tensor_tensor(U[0:64, 2 * CW:3 * CW],
                                        xp0415t[0:64, CW:2 * CW],
                                        attE[0:64, :], AL.mult)
                nc.gpsimd.tensor_tensor(U[0:64, 3 * CW:4 * CW],
                                        xp23rt[0:64, 0:CW],
                                        attE[0:64, :], AL.mult)
                nc.gpsimd.tensor_tensor(U[0:64, 4 * CW:5 * CW],
                                        xp23rt[0:64, CW:2 * CW],
                                        attE[0:64, :], AL.mult)
                nc.gpsimd.tensor_tensor(L[64:128, CW:2 * CW],
                                        xp0415t[64:128, 0:CW],
                                        attE[64:128, :], AL.mult)
                nc.gpsimd.tensor_tensor(L[64:128, 2 * CW:3 * CW],
                                        xp0415t[64:128, CW:2 * CW],
                                        attE[64:128, :], AL.mult)
                return dict(U=U, L=L, xhP=xhP, c0=c0, c1=c1)

            def stageC(pctx):
                U, L, xhP = pctx["U"], pctx["L"], pctx["xhP"]
                c0, c1 = pctx["c0"], pctx["c1"]

                # ------------- decomposition conv1 + relu -------------
                p_du = pp.tile([128, CW], F32, tag="ps", name="p_du")
                mm(p_du, S_DU, 128, U[:, 0:CW], True, True)
                p_dl = pp.tile([128, CW], F32, tag="ps", name="p_dl")
                mm(p_dl, S_DL, 128, L[:, 0:CW], True, True)
                H_du = hp.tile([128, CW], BF16, tag="Hdu", name="Hdu")
                nc.scalar.activation(H_du[:, :], p_du[:, :], AF.Relu,
                                     bias=bias(BV_D1))
                H_dl = hp.tile([128, CW], BF16, tag="Hdl", name="Hdl")
                nc.scalar.activation(H_dl[:, :], p_dl[:, :], AF.Relu,
                                     bias=bias(BV_D1))

                # ------------- composition blocks -------------
                # ALL conv1 passes first (H evacs drain on Act/DVE behind
                # them), then every conv2/z pass — keeps PE stall-free
                zt = [None] * 5
                Hc = {}
                for i, zi, role in BLOCK_SEQ:
                    up = i < 4
                    if up:
                        mv = U[:, (1 + i) * CW:(2 + i) * CW]
                        s1, bH = S_CU, BV_U1
                    else:
                        mv = L[:, (i - 3) * CW:(i - 2) * CW]
                        s1, bH = S_CL, BV_L1
                    p_c = pp.tile([128, CW], F32, tag="ps", name=f"pc{i}")
                    mm(p_c, s1, 128, mv, True, True)
                    H_c = hp.tile([128, CW], BF16, tag=f"Hc{i}", name=f"Hc{i}")
                    if i in (3, 4):
                        nc.vector.tensor_scalar(H_c[:, :], p_c[:, :],
                                                bias(bH), 0.0, AL.add, AL.max)
                    else:
                        nc.scalar.activation(H_c[:, :], p_c[:, :], AF.Relu,
                                             bias=bias(bH))
                    Hc[i] = H_c

                # z0 = conv2(dec pair)
                z0p = pp.tile([128, CW], F32, tag="ps", name="z0p")
                mm(z0p, S_DW2H0, 128, H_du[:, :], True, False)
                mm(z0p, S_DW2H1, 128, H_dl[:, :], False, True)

                zpsum = {}
                for i, zi, role in BLOCK_SEQ:
                    up = i < 4
                    if role != 1:
                        zpsum[zi] = pp.tile([128, CW], F32, tag="ps",
                                            name=f"zp{zi}")
                    mm(zpsum[zi], W2_STAT[(up, role == 1)], 128, Hc[i][:, :],
                       role != 1, role != 0)
                    if role == 1:
                        zt[zi] = zp.tile([128, CW], BF16, tag=f"zt{zi}",
                                         name=f"zt{zi}")
                        if zi == 1:
                            nc.scalar.activation(zt[zi][:, :], zpsum[zi][:, :],
                                                 AF.Relu)
                        else:
                            nc.vector.tensor_scalar(zt[zi][:, :],
                                                    zpsum[zi][:, :], 0.0,
                                                    0.0, AL.add, AL.max)

                # ------------- message sum -------------
                # z biases are already in the psums, so the two single-z
                # relus fuse into the adds: (zp max 0) add acc
                zt[0] = zp.tile([128, CW], BF16, tag="zt0", name="zt0")
                nc.vector.tensor_scalar(zt[0][:, :], z0p[:, :], 0.0,
                                        0.0, AL.add, AL.max)
                m01 = zp.tile([128, CW], BF16, tag="m01")
                nc.gpsimd.tensor_tensor(m01[:, :], zt[0][:, :], zt[1][:, :],
                                        AL.add)
                m23 = zp.tile([128, CW], BF16, tag="m23")
                nc.vector.scalar_tensor_tensor(m23[:, :], zpsum[3][:, :], 0.0,
                                               zt[2][:, :], AL.max, AL.add)
                m03 = zp.tile([128, CW], BF16, tag="m03")
                nc.gpsimd.tensor_tensor(m03[:, :], m01[:, :], m23[:, :],
                                        AL.add)
                M = zp.tile([128, CW], BF16, tag="M", bufs=3)
                nc.vector.scalar_tensor_tensor(M[:, :], zpsum[4][:, :], 0.0,
                                               m03[:, :], AL.max, AL.add)
                return dict(M=M, xhP=xhP, c0=c0, c1=c1)

            def stageB1(ctx):
                M, xhP = ctx["M"], ctx["xhP"]

                # ------------- GRU r/u gates + reset premult -------------
                # xh pass first: it needs no M, so only the stop pass
                # waits on the message sum
                p_r = pp.tile([128, CW], F32, tag="ps", name="p_r")
                mm(p_r, S_R_H, 128, xhP[:, :], True, False)
                mm(p_r, S_R_M, 128, M[:, :], False, True)
                p_u = pp.tile([128, CW], F32, tag="ps", name="p_u")
                mm(p_u, S_U_H, 128, xhP[:, :], True, False)
                mm(p_u, S_U_M, 128, M[:, :], False, True)
                Rt = gpl.tile([128, CW], BF16, tag="Rt", bufs=3)
                nc.scalar.activation(Rt[:, :], p_r[:, :], AF.Sigmoid,
                                     bias=bias(BV_R))
                Ut = gpl.tile([128, CW], BF16, tag="Ut", bufs=3)
                nc.scalar.activation(Ut[:, :], p_u[:, :], AF.Sigmoid,
                                     bias=bias(BV_U))
                rhM = gpl.tile([128, CW], BF16, tag="rhM", bufs=3)
                nc.gpsimd.tensor_tensor(rhM[:, :], Rt[:, :], xhP[:, :],
                                        AL.mult)
                ctx["Ut"] = Ut
                ctx["rhM"] = rhM
                return ctx

            def stageB2(ctx):
                M, xhP, c0, c1 = ctx["M"], ctx["xhP"], ctx["c0"], ctx["c1"]
                Ut, rhM = ctx["Ut"], ctx["rhM"]
                # ------------- GRU candidate + combine -------------
                p_cc = pp.tile([128, CW], F32, tag="ps", name="p_cc")
                mm(p_cc, S_C_M, 128, M[:, :], True, False)
                mm(p_cc, S_C_RH, 128, rhM[:, :], False, True)
                Ct = gpl.tile([128, CW], BF16, tag="Ct")
                nc.scalar.activation(Ct[:, :], p_cc[:, :], AF.Tanh,
                                     bias=bias(BV_C))
                # out = h + u*(c - h)
                Dt = gpl.tile([128, CW], BF16, tag="Dt")
                nc.gpsimd.tensor_tensor(Dt[:, :], Ct[:, :], xhP[:, :],
                                        AL.subtract)
                Et = gpl.tile([128, CW], BF16, tag="Et")
                nc.vector.tensor_tensor(Et[:, :], Ut[:, :], Dt[:, :], AL.mult)
                outT = gpl.tile([128, CW], BF16, tag="outT")
                nc.gpsimd.tensor_tensor(outT[:, :], xhP[:, :], Et[:, :],
                                        AL.add)

                nc.sync.dma_start(out=out[:, c0:c1], in_=outT[:, :])

            # software pipeline: P1+P2 (loads/premults) 2 chunks ahead,
            # C (convs) 1 ahead, B1 (r/u gates) current, B2 (candidate +
            # combine + store) one behind — the GRU tail never blocks the
            # next chunk's H evacuations on the Act queue.
            p1x = [None] * NCHUNK
            p2x = [None] * NCHUNK
            ccx = [None] * NCHUNK
            p1x[0] = stageP1(0)
            nc.sync.dma_start(out=attT[:, CW:], in_=att[:, CW:])
            p2x[0] = stageP2(p1x[0])
            p1x[1] = stageP1(1)
            p2x[1] = stageP2(p1x[1])
            ccx[0] = stageC(p2x[0])
            for j in range(NCHUNK):
                if j + 2 < NCHUNK:
                    p1x[j + 2] = stageP1(j + 2)
                    p2x[j + 2] = stageP2(p1x[j + 2])
                if j + 1 < NCHUNK:
                    ccx[j + 1] = stageC(p2x[j + 1])
                stageB1(ccx[j])
                if j > 0:
                    stageB2(ccx[j - 1])
            stageB2(ccx[NCHUNK - 1])

    nc.compile()
    return nc


def _fold(W, p):
    g, b, m, v = p[0], p[1], p[2], p[3]
    s = g / np.sqrt(v + EPS)
    return (s[:, None] * W).astype(np.float32), (b - m * s).astype(np.float32)


RC = None


def _rc():
    global RC
    if RC is None:
        ci = np.arange(HD)
        RC = np.stack([10 * g + ci for g in range(G)])  # [6, 10]
    return RC


def _build_params(dW1, dbn1, dW2, dbn2, uW1, ubn1, uW2, ubn2,
                  lW1, lbn1, lW2, lbn2, guWg, gubg, guWc, gubc,
                  glWg, glbg, glWc, glbc):
    dW1f, bd1 = _fold(dW1, dbn1)
    dW2f, bd2 = _fold(dW2, dbn2)
    uW1f, bu1 = _fold(uW1, ubn1)
    uW2f, bu2 = _fold(uW2, ubn2)
    lW1f, bl1 = _fold(lW1, lbn1)
    lW2f, bl2 = _fold(lW2, lbn2)

    S = np.zeros((NS, 128, 128), np.float32)
    rc = _rc()

    def quad(idx, m_off, o_off, Wblk):
        # S[moving_row, out_row] = W[out_ch, in_ch] per group
        for g in range(G):
            S[np.ix_([idx], m_off + rc[g], o_off + rc[g])] = Wblk.T[None]

    # attention broadcasts: S_UL moving rows 6k+g (p_att plane k+1)
    for g in range(G):
        for k in range(4):
            S[S_UL, 6 * k + g, rc[g]] = 1.0
        for k in (4, 5):
            S[S_UL, 6 * k + g, H1 + rc[g]] = 1.0
        # S_A12: moving = att rows 32:48, stationary slice at base 32
        S[S_A12, 36 + g, rc[g]] = 1.0       # h_att1 (att rows 36:42)
        S[S_A12, 42 + g, H1 + rc[g]] = 1.0  # h_att2 (att rows 42:48)

    # dec conv1: U tile = [xfm_u (cat 0:10) ; xh_u (cat 10:20)]
    quad(S_DU, 0, 0, dW1f[0:10, 0:10])
    quad(S_DU, 0, H1, dW1f[10:20, 0:10])
    quad(S_DU, H1, 0, dW1f[0:10, 10:20])
    quad(S_DU, H1, H1, dW1f[10:20, 10:20])
    # dec conv1 lower: L tile = [xh_l (cat 10:20) ; xfm_l (cat 0:10)]
    quad(S_DL, 0, 0, dW1f[0:10, 10:20])
    quad(S_DL, 0, H1, dW1f[10:20, 10:20])
    quad(S_DL, H1, 0, dW1f[0:10, 0:10])
    quad(S_DL, H1, H1, dW1f[10:20, 0:10])
    # comp conv1 upper: U tile = [xpm (cat 10:20) ; xh_u (cat 0:10)]
    quad(S_CU, 0, 0, uW1f[0:10, 10:20])
    quad(S_CU, 0, H1, uW1f[10:20, 10:20])
    quad(S_CU, H1, 0, uW1f[0:10, 0:10])
    quad(S_CU, H1, H1, uW1f[10:20, 0:10])
    # comp conv1 lower: L tile = [xh_l (cat 0:10) ; xpm (cat 10:20)]
    quad(S_CL, 0, 0, lW1f[0:10, 0:10])
    quad(S_CL, 0, H1, lW1f[10:20, 0:10])
    quad(S_CL, H1, 0, lW1f[0:10, 10:20])
    quad(S_CL, H1, H1, lW1f[10:20, 10:20])

    def conv2(ih0, ih1, Wf):
        for idx, off in ((ih0, 0), (ih1, H1)):
            quad(idx, 0, off, Wf[:, 0:10])
            quad(idx, H1, off, Wf[:, 10:20])

    conv2(S_DW2H0, S_DW2H1, dW2f)
    conv2(S_UW2H0, S_UW2H1, uW2f)
    conv2(S_LW2H0, S_LW2H1, lW2f)
    # z biases ride stationary row 60 (H's ones-row), once per psum:
    # first/single passes carry the upper-col bias, second passes lower
    for g in range(G):
        S[S_DW2H0, 60, rc[g]] = bd2
        S[S_DW2H1, 60, H1 + rc[g]] = bd2
        S[S_UW2H0, 60, rc[g]] = bu2
        S[S_LW2H1, 60, H1 + rc[g]] = bl2

    # gates: M pair / xh pair moving; upper block -> cols 0:60, lower -> 64:
    quad(S_R_M, 0, 0, guWg[0:10, 0:10])
    quad(S_R_M, H1, H1, glWg[0:10, 0:10])
    quad(S_R_H, 0, 0, guWg[0:10, 10:20])
    quad(S_R_H, H1, H1, glWg[0:10, 10:20])
    quad(S_U_M, 0, 0, guWg[10:20, 0:10])
    quad(S_U_M, H1, H1, glWg[10:20, 0:10])
    quad(S_U_H, 0, 0, guWg[10:20, 10:20])
    quad(S_U_H, H1, H1, glWg[10:20, 10:20])
    quad(S_C_M, 0, 0, guWc[:, 0:10])
    quad(S_C_M, H1, H1, glWc[:, 0:10])
    quad(S_C_RH, 0, 0, guWc[:, 10:20])
    quad(S_C_RH, H1, H1, glWc[:, 10:20])

    bvec = np.zeros((128, NB), np.float32)

    def setb(col, top, bot):
        for g in range(G):
            bvec[rc[g], col] = top
            bvec[H1 + rc[g], col] = bot

    setb(BV_D1, bd1[0:10], bd1[10:20])
    setb(BV_U1, bu1[0:10], bu1[10:20])
    setb(BV_L1, bl1[0:10], bl1[10:20])
    # z biases live in the conv2 stationaries (H ones-row); BV_Z* = 0.
    # The H-bias columns get 1.0 at pad row 60 to produce that ones-row.
    bvec[60, [BV_D1, BV_U1, BV_L1]] = 1.0
    setb(BV_R, gubg[0:10], glbg[0:10])
    setb(BV_U, gubg[10:20], glbg[10:20])
    setb(BV_C, gubc, glbc)

    # flatten stationaries to [128, NS*128]
    Sflat = np.ascontiguousarray(np.transpose(S, (1, 0, 2)).reshape(128, NS * 128))
    return Sflat, bvec


_NC_CACHE = None


def _get_nc():
    global _NC_CACHE
    if _NC_CACHE is None:
        _NC_CACHE = _build_nc()
    return _NC_CACHE


def _planar(a):
    # [..., HD, H, W] -> [..., 64, GP] zero-padded planar
    lead = a.shape[:-3]
    a = np.asarray(a, np.float32).reshape(lead + (HD, G, GP))
    a = np.moveaxis(a, -2, -3)
    a = a.reshape(lead + (60, GP))
    pad = np.zeros(lead + (4, GP), np.float32)
    return np.concatenate([a, pad], axis=-2)


def _unplanar(a):
    # [..., 60, GP] -> [..., HD, H, W]
    lead = a.shape[:-2]
    a = a.reshape(lead + (G, HD, GP))
    a = np.moveaxis(a, -3, -2)
    return a.reshape(lead + (HD, 192, 192))


BF_NP = mybir.dt.np(mybir.dt.bfloat16)


def make_in_maps(xf, xh, xp, h_att, p_att, smats, bvecs):
    smatsB = smats.astype(BF_NP)
    in_maps = []
    for b in range(B):
        xhP = _planar(xh[:, b]).reshape(128, GP)
        xfP = _planar(xf[b])
        xpP = _planar(xp[:, b])
        # upper xh half, 5 adjacent column replicas per chunk
        xhu = np.repeat(xhP[0:64].reshape(64, NCHUNK, 1, CW), 5,
                        axis=2).reshape(64, 5 * GP)
        att = np.asarray(p_att[1:7, b, 0], np.float32).reshape(36, GP)
        a12 = np.zeros((128, GP), np.float32)
        a12[0:60] = np.repeat(np.asarray(h_att[1, b, 0], np.float32)
                              .reshape(G, 1, GP), HD, axis=1).reshape(60, GP)
        a12[H1:H1 + 60] = np.repeat(np.asarray(h_att[2, b, 0], np.float32)
                                    .reshape(G, 1, GP), HD, axis=1).reshape(60, GP)
        in_maps.append(dict(
            xh=np.ascontiguousarray(xhP).astype(BF_NP),
            xp0415=np.ascontiguousarray(
                np.stack([np.concatenate([xpP[0], xpP[4]], axis=0)
                          .reshape(128, NCHUNK, CW),
                          np.concatenate([xpP[1], xpP[5]], axis=0)
                          .reshape(128, NCHUNK, CW)], axis=2)
                .reshape(128, 2 * GP)).astype(BF_NP),
            xp23r=np.ascontiguousarray(
                np.stack([xpP[2][0:64].reshape(64, NCHUNK, CW),
                          xpP[3][0:64].reshape(64, NCHUNK, CW)], axis=2)
                .reshape(64, 2 * GP)).astype(BF_NP),
            xf2=np.ascontiguousarray(
                np.concatenate([xfP, xfP], axis=0)).astype(BF_NP),
            xhu5=np.ascontiguousarray(xhu).astype(BF_NP),
            att=np.ascontiguousarray(att).astype(BF_NP),
            a12e=np.ascontiguousarray(a12).astype(BF_NP),
            smats=smatsB,
            bvecs=bvecs,
        ))
    return in_maps


def kernel(xf, xh, xp, h_att, p_att,
           dW1, dbn1, dW2, dbn2,
           uW1, ubn1, uW2, ubn2,
           lW1, lbn1, lW2, lbn2,
           guWg, gubg, guWc, gubc,
           glWg, glbg, glWc, glbc,
           _trace=False):
    from concourse.bass_utils import run_bass_kernel_spmd

    args = [np.asarray(a, dtype=np.float32) for a in
            (dW1, dbn1, dW2, dbn2, uW1, ubn1, uW2, ubn2,
             lW1, lbn1, lW2, lbn2, guWg, gubg, guWc, gubc,
             glWg, glbg, glWc, glbc)]
    smats, bvecs = _build_params(*args)
    in_maps = make_in_maps(np.asarray(xf, np.float32), np.asarray(xh, np.float32),
                           np.asarray(xp, np.float32),
                           np.asarray(h_att, np.float32),
                           np.asarray(p_att, np.float32), smats, bvecs)

    nc = _get_nc()
    res = run_bass_kernel_spmd(nc, in_maps, core_ids=list(range(B)),
                               trace=_trace)
    out = np.empty((2, B, HD, 192, 192), np.float32)
    for b in range(B):
        o = np.asarray(res.results[b]["out"], np.float32)
        out[0, b] = _unplanar(o[0:60])
        out[1, b] = _unplanar(o[H1:H1 + 60])
    if _trace:
        return out, res
    return out
